# revision 16
# baseline (speedup 1.0000x reference)
"""Multi-head attention kernel for 8 TRN2 NeuronCores.

Problem: x(4,2048,1024) -> MHA(16 heads, d=64) -> out(4,2048,1024), f32.

Sharding: core c handles (batch b = c//2, seq half = c%2): it computes
attention outputs (incl. all projections) for its 1024 query rows over all 16
heads.  K/V projections for the full batch are computed locally per core (2x
redundant) which keeps cores fully independent - zero collectives.

v5 highlights:

* All projections run in bf16 (x and weights cast on host).
* Scores run in fp8e4 with the DoubleRow perf mode (0.5 cycles/moving-row):
  the pair dim carries a DUAL-fp8 decomposition of Q: pair0 = (q_hi, k8),
  pair1 = (q_lo, k8/4) where q_lo = fp8(4*(q - q_hi)), so the PE computes
  k8^T (q_hi + q_lo/4) - near-bf16 Q precision at fp8 speed.  K stays
  single-fp8 (~7e-3 rel err end to end).
* Softmax exp runs on ScalarE out of PSUM, writing bf16.  ScalarE is the
  bottleneck engine (~266us busy), so the schedule keeps its queue non-empty
  from the first scores (~16us) to the end.
* PV runs in bf16 in the SWAPPED orientation: ov[q,65] = E^T @ [V | 1];
  column 64 (ones) gives the softmax denominator per q ROW -> normalization
  is a partition-local reciprocal + tensor_scalar_mul.  The normalized
  [128q, 64d] tile is PE-transposed (bf16) back into att_sb[emb, q] for the
  bf16 output projection.
* Q and V never touch DRAM: Q-projection evacs write dual-fp8 directly into
  the next pair's qtp SBUF tile; the V projection is computed
  head-pair-column-major into SBUF-resident vcb tiles laid out as
  [keys, rc x (64 V cols even | 1.0 | 64 V cols odd | 1.0)] which the PV
  matmuls read directly.
* Software pipelining: per head-pair iteration the emission interleaves this
  block's scores (early jc2 first, matching ScalarE's drain rate) with the
  next pair's K/V/Q projection blocks, then emits the PREVIOUS block's
  PV/normalize/transpose, so the in-order PE always has work that does not
  wait on ScalarE.  The output projection is split into slices overlapped
  with the last pair's attention and the final PV chunks.

This walrus build accepts only ONE sync-wait per instruction, so a post-pass
splits multi-wait instructions into single-wait NoOps (_split_multi_waits).
"""

import numpy as np
from contextlib import ExitStack

P = 128
EMB = 1024
SEQ = 2048
QR = 1024          # query rows per core
NH = 16
HD = 64
EC = EMB // P      # 8 contraction chunks
RC = SEQ // P      # 16 seq row chunks
NB = 512           # free-dim block
SCALE = 0.125      # 1/sqrt(64)

_COMPILED = None


def _patch_tile_drain():
    """This walrus build only accepts ONE sync-wait per Drain instruction; the
    stock TileContext tail drain carries one wait per pending proc.  Split it
    into a chain of single-wait drains."""
    import concourse.tile as tile
    from concourse.vector_clock import ScopedClock, VectorClock

    if getattr(tile.TileContext, "_drain_patched", False):
        return

    def _drain_and_barrier(self, tick_clock, wait_clock):
        nc = self.nc
        gc = tick_clock.global_clock
        vals = eval(repr(gc).replace("VectorClock", ""))
        n = len(vals)
        for i, v in enumerate(vals):
            if v > 0:
                sub = VectorClock([vals[j] if j == i else 0 for j in range(n)])
                d = nc.sync.drain()
                wait_clock.add_sem_waits(d.ins, ScopedClock({None: sub}))
        nc.all_engine_barrier()
        popped = nc._tile_sem_poison_stack.pop()
        assert popped is self._sem_poison
        nc.clear_and_free_semaphores(list(self.sems.allocated().values()))
        nc.all_engine_barrier()

    tile.TileContext._drain_and_barrier = _drain_and_barrier
    tile.TileContext._drain_patched = True


def _build():
    import concourse.bass as bass
    import concourse.mybir as mybir
    import concourse.tile as tile

    _patch_tile_drain()

    f32 = mybir.dt.float32
    f8 = mybir.dt.float8e4
    bf16 = mybir.dt.bfloat16
    Exp = mybir.ActivationFunctionType.Exp
    DR = mybir.MatmulPerfMode.DoubleRow
    Sub = mybir.AluOpType.subtract

    nc = bass.Bass()

    # xt holds this core's batch transposed, with the core's 1024 query rows
    # FIRST (host pre-permutes; key/value row order is irrelevant to MHA).
    xt = nc.dram_tensor("xt", [EMB, SEQ], bf16, kind="ExternalInput")
    wqt = nc.dram_tensor("wqt", [EMB, EMB], bf16, kind="ExternalInput")
    wkt = nc.dram_tensor("wkt", [EMB, EMB], bf16, kind="ExternalInput")
    wvt = nc.dram_tensor("wvt", [EMB, EMB], bf16, kind="ExternalInput")
    wot = nc.dram_tensor("wot", [EMB, EMB], bf16, kind="ExternalInput")
    bqp = nc.dram_tensor("bqp", [P, EC], f32, kind="ExternalInput")
    bkp = nc.dram_tensor("bkp", [P, EC], f32, kind="ExternalInput")
    bob = nc.dram_tensor("bob", [P, EMB], f32, kind="ExternalInput")
    idn = nc.dram_tensor("idn", [P, P], bf16, kind="ExternalInput")
    out = nc.dram_tensor("out", [QR, EMB], f32, kind="ExternalOutput")

    with tile.TileContext(nc) as tc, ExitStack() as ctx:
        big = ctx.enter_context(tc.tile_pool(name="big", bufs=1))
        wpool = ctx.enter_context(tc.tile_pool(name="w", bufs=1))
        pspool = ctx.enter_context(tc.tile_pool(name="ps", bufs=2, space="PSUM"))
        stpool = ctx.enter_context(tc.tile_pool(name="st", bufs=2, space="PSUM"))
        ovpool = ctx.enter_context(tc.tile_pool(name="ov", bufs=2, space="PSUM"))
        evac = ctx.enter_context(tc.tile_pool(name="evac", bufs=3))
        ptpool = ctx.enter_context(tc.tile_pool(name="pt", bufs=3))
        kpool = ctx.enter_context(tc.tile_pool(name="kp", bufs=2))
        wkpool = ctx.enter_context(tc.tile_pool(name="wk", bufs=2))
        qpool = ctx.enter_context(tc.tile_pool(name="qp", bufs=2))
        vpool = ctx.enter_context(tc.tile_pool(name="vp", bufs=3))
        nrm = ctx.enter_context(tc.tile_pool(name="nrm", bufs=2))
        misc = ctx.enter_context(tc.tile_pool(name="misc", bufs=1))

        # ---- persistent loads (critical-path order) ---------------------
        bq_sb = misc.tile([P, EC], f32, tag="bq")
        nc.sync.dma_start(bq_sb[:], bqp[:])
        bk_sb = misc.tile([P, EC], f32, tag="bk")
        nc.sync.dma_start(bk_sb[:], bkp[:])

        def load_wk(t):
            wk_t = wkpool.tile([P, EC * P], bf16, tag="wk", name="wk_t")
            for ec in range(EC):
                nc.sync.dma_start(
                    wk_t[:, ec * P:(ec + 1) * P],
                    wkt[ec * P:(ec + 1) * P, t * P:(t + 1) * P])
            return wk_t

        wk_tiles = {0: load_wk(0)}
        # wq and x query columns interleaved per ec chunk so the first
        # Q-projection chain pipelines behind the DMA
        wq_sb = wpool.tile([P, EC * EMB], bf16, tag="w", name="wq_sb")
        xt_sb = big.tile([P, EC * SEQ], bf16, tag="xt")
        for ec in range(EC):
            nc.sync.dma_start(wq_sb[:, ec * EMB:(ec + 1) * EMB],
                              wqt[ec * P:(ec + 1) * P, :])
            nc.sync.dma_start(xt_sb[:, ec * SEQ: ec * SEQ + QR],
                              xt[ec * P:(ec + 1) * P, 0:QR])
        for ec in range(EC):
            nc.sync.dma_start(xt_sb[:, ec * SEQ + QR: (ec + 1) * SEQ],
                              xt[ec * P:(ec + 1) * P, QR:SEQ])

        # ---- building blocks --------------------------------------------
        def qp_tile(oc, ib, qtp_dst):
            """Q projection for output chunk oc (= head pair), query block ib,
            written as dual-fp8 (q_hi at [ib*NB], q_lo at [QR+ib*NB])
            directly into the pair's qtp SBUF tile."""
            ps = pspool.tile([P, NB], f32, tag="ps", name="ps")
            for ec in range(EC):
                nc.tensor.matmul(
                    ps[:],
                    wq_sb[:, ec * EMB + oc * P: ec * EMB + (oc + 1) * P],
                    xt_sb[:, ec * SEQ + ib * NB: ec * SEQ + (ib + 1) * NB],
                    start=(ec == 0), stop=(ec == EC - 1))
            qf = evac.tile([P, NB], f32, tag="qf", name="qf")
            nc.vector.tensor_scalar_add(qf[:], ps[:], bq_sb[:, oc:oc + 1])
            nc.vector.tensor_copy(qtp_dst[:, ib * NB:(ib + 1) * NB], qf[:])
            qd = evac.tile([P, NB], f32, tag="qd", name="qd")
            nc.vector.tensor_tensor(
                qd[:], qf[:], qtp_dst[:, ib * NB:(ib + 1) * NB], Sub)
            nc.vector.tensor_scalar_mul(
                qtp_dst[:, QR + ib * NB: QR + (ib + 1) * NB], qd[:], 4.0)

        def kp_block(ktp, t, jb):
            """K projection for head pair t, key block jb -> fp8 K in
            ktp[:, 0:SEQ] and fp8 K/4 in ktp[:, SEQ:2SEQ] (DoubleRow pair
            partner for the dual-fp8 Q correction term)."""
            ps = pspool.tile([P, NB], f32, tag="ps", name="ps")
            for ec in range(EC):
                nc.tensor.matmul(
                    ps[:],
                    wk_tiles[t][:, ec * P:(ec + 1) * P],
                    xt_sb[:, ec * SEQ + jb * NB: ec * SEQ + (jb + 1) * NB],
                    start=(ec == 0), stop=(ec == EC - 1))
            nc.vector.tensor_scalar_add(
                ktp[:, jb * NB:(jb + 1) * NB], ps[:], bk_sb[:, t:t + 1])
            nc.vector.tensor_scalar(
                ktp[:, SEQ + jb * NB: SEQ + (jb + 1) * NB], ps[:],
                bk_sb[:, t:t + 1], 0.25,
                mybir.AluOpType.add, mybir.AluOpType.mult)

        def vp_block(vcb, t, rc):
            """V projection for head pair t, key row chunk rc, written into
            the SBUF-resident [V_even | 1 | V_odd | 1] layout."""
            ps = pspool.tile([P, P], f32, tag="ps", name="ps")
            for ec in range(EC):
                nc.tensor.matmul(
                    ps[:],
                    xt_sb[:, ec * SEQ + rc * P: ec * SEQ + (rc + 1) * P],
                    wv_sb[:, ec * EMB + t * P: ec * EMB + (t + 1) * P],
                    start=(ec == 0), stop=(ec == EC - 1))
            nc.vector.tensor_copy(vcb[:, rc * 130: rc * 130 + HD], ps[:, 0:HD])
            nc.vector.tensor_copy(vcb[:, rc * 130 + 65: rc * 130 + 65 + HD],
                                  ps[:, HD:P])

        def new_vcb():
            vcb = vpool.tile([P, RC * 130], bf16, tag="vcb", name="vcb")
            for e in range(2):
                ones_ap = bass.AP(vcb.tensor, vcb.offset + e * 65 + HD,
                                  [list(vcb.ap[0]), [130, RC]])
                nc.vector.memset(ones_ap, 1.0)
            return vcb

        def op_slice(rc8):
            for ob in range(2):
                ps = pspool.tile([P, NB], f32, tag="ps", name="ps")
                for cc in range(EC):
                    nc.tensor.matmul(
                        ps[:],
                        att_sb[:, cc * QR + rc8 * P: cc * QR + (rc8 + 1) * P],
                        wot_sb[:, cc * EMB + ob * NB: cc * EMB + (ob + 1) * NB],
                        start=(cc == 0), stop=(cc == EC - 1))
                ev = evac.tile([P, NB], f32, tag="evo", name="ev")
                nc.vector.tensor_add(
                    ev[:], ps[:], bob_sb[:, ob * NB:(ob + 1) * NB])
                nc.gpsimd.dma_start(
                    out[rc8 * P:(rc8 + 1) * P, ob * NB:(ob + 1) * NB], ev[:])

        def pair_ap(t2, pstart, pcount, off, pair_stride, n):
            """[pcount partitions from pstart] x (2 pairs) x (n contiguous)."""
            sl = t2[pstart:pstart + pcount]
            return bass.AP(sl.tensor, sl.offset + off,
                           [list(sl.ap[0]), [pair_stride, 2], [1, n]])

        def score_block(ktp, qtp, pt, e, ib, jc2):
            # priority boost: the scheduler should always prefer producing
            # scores (and their exps) over projection/PV filler work, so the
            # bottleneck ScalarE never starves
            with tc.high_priority(offset=200):
                st_ps = stpool.tile([P, 2 * NB], f32, tag="st")
                for u in range(2):
                    jc = jc2 * 2 + u
                    nc.tensor.matmul(
                        st_ps[:, u * NB:(u + 1) * NB],
                        pair_ap(ktp, e * HD, HD, jc * P, SEQ, P),
                        pair_ap(qtp, e * HD, HD, ib * NB, QR, NB),
                        start=True, stop=True, perf_mode=DR)
                nc.scalar.activation(pt[:, jc2 * 2 * NB:(jc2 + 1) * 2 * NB],
                                     st_ps[:], Exp, scale=SCALE)

        pendings = []

        def pv_chain(t, e, ib, pt, vcb, qc):
            ov = ovpool.tile([P, 65], f32, tag="ov", name="ov")
            for rc in range(RC):
                jc2, u = rc // 2, rc % 2
                nc.tensor.matmul(
                    ov[:],
                    pt[:, jc2 * 2 * NB + u * NB + qc * P:
                       jc2 * 2 * NB + u * NB + (qc + 1) * P],
                    vcb[:, rc * 130 + e * 65: rc * 130 + (e + 1) * 65],
                    start=(rc == 0), stop=(rc == RC - 1))
            rs = nrm.tile([P, 1], f32, tag="rs")
            nc.vector.reciprocal(rs[:], ov[:, HD:65])
            atmp = nrm.tile([P, HD], bf16, tag="atmp")
            nc.vector.tensor_scalar_mul(atmp[:], ov[:, 0:HD], rs[:, 0:1])
            trp = ovpool.tile([HD, P], bf16, tag="ov", name="trp")
            nc.tensor.transpose(trp[:], atmp[:], idn_sb[:])
            nc.vector.tensor_copy(
                att_sb[e * HD:(e + 1) * HD,
                       t * QR + ib * NB + qc * P:
                       t * QR + ib * NB + (qc + 1) * P],
                trp[:])

        def emit_pv():
            t, e, ib, pt, vcb = pendings.pop(0)
            for qc in range(NB // P):
                pv_chain(t, e, ib, pt, vcb, qc)

        # ---- preamble: warm the PE p-state during the DMA prefetch ------
        # (a cold PE runs at half clock for its first 3us of continuous
        # work; junk matmuls during the ~13us weight/x prefetch window make
        # the first projections run at full clock)
        wtile = misc.tile([P, NB], bf16, tag="warm")
        nc.vector.memset(wtile[:], 1.0)
        for _ in range(44):
            ps = pspool.tile([P, NB], f32, tag="ps", name="ps")
            nc.tensor.matmul(ps[:], wtile[:, 0:P], wtile[:], start=True,
                             stop=True)

        # ---- preamble: pair 0's inputs ----------------------------------
        qtp_cur = qpool.tile([P, 2 * QR], f8, tag="qt", name="qtp")
        qp_tile(0, 0, qtp_cur)
        ktp_cur = kpool.tile([P, 2 * SEQ], f8, tag="kt", name="ktp")
        kp_block(ktp_cur, 0, 0)
        kp_block(ktp_cur, 0, 1)
        qp_tile(0, 1, qtp_cur)
        kp_block(ktp_cur, 0, 2)
        kp_block(ktp_cur, 0, 3)

        # non-critical loads: wv, out-proj bias, identity
        wv_sb = big.tile([P, EC * EMB], bf16, tag="wv", name="wv_sb")
        for ec in range(EC):
            nc.sync.dma_start(wv_sb[:, ec * EMB:(ec + 1) * EMB],
                              wvt[ec * P:(ec + 1) * P, :])
        bob_sb = misc.tile([P, EMB], f32, tag="bob")
        nc.sync.dma_start(bob_sb[:], bob[:])
        idn_sb = misc.tile([P, P], bf16, tag="idn")
        nc.sync.dma_start(idn_sb[:], idn[:])

        # pair 0's V projection is NOT done here: its 16 chains would sit on
        # the PE critical path before the first scores and delay ScalarE's
        # start by ~7us.  It is interleaved into pair 0's first two blocks
        # instead (PV needs it only from the second block's end).
        vcb_cur = new_vcb()

        att_sb = big.tile([P, EC * QR], bf16, tag="att")
        wot_sb = None

        def load_w_ot():
            w_sb = wpool.tile([P, EC * EMB], bf16, tag="w", name="w_sb")
            for ec in range(EC):
                nc.sync.dma_start(w_sb[:, ec * EMB:(ec + 1) * EMB],
                                  wot[ec * P:(ec + 1) * P, :])
            return w_sb

        # ---- main loop: 8 head pairs, fully pipelined -------------------
        last = NH // 2 - 1
        for t in range(NH // 2):
            ktp, qtp, vcb = ktp_cur, qtp_cur, vcb_cur
            if t < last:
                wk_tiles[t + 1] = load_wk(t + 1)
                ktp_cur = kpool.tile([P, 2 * SEQ], f8, tag="kt", name="ktp")
                vcb_cur = new_vcb()
                qtp_cur = qpool.tile([P, 2 * QR], f8, tag="qt", name="qtp")
            ebs = ([(e, ib) for e in range(2) for ib in range(QR // NB)]
                   if t < last else
                   [(e, ib) for ib in range(QR // NB) for e in range(2)])
            for idx, (e, ib) in enumerate(ebs):
                h = 2 * t + e
                pt = ptpool.tile([P, RC * NB], bf16, tag="pt")
                # scores early (refill ScalarE), next-pair projections
                # interleaved at ScalarE's drain rate
                score_block(ktp, qtp, pt, e, ib, 0)
                score_block(ktp, qtp, pt, e, ib, 1)
                if t < last:
                    kp_block(ktp_cur, t + 1, idx)
                score_block(ktp, qtp, pt, e, ib, 2)
                score_block(ktp, qtp, pt, e, ib, 3)
                vp0 = ([(0, rc) for rc in range(6 * idx, min(6 * idx + 6, RC))]
                       if t == 0 and idx < 3 else [])
                if vp0:
                    # pair 0's own V projection (deferred from the preamble)
                    for _, rc in vp0[:3]:
                        vp_block(vcb, 0, rc)
                if t < last:
                    vp_block(vcb_cur, t + 1, 4 * idx)
                    vp_block(vcb_cur, t + 1, 4 * idx + 1)
                score_block(ktp, qtp, pt, e, ib, 4)
                score_block(ktp, qtp, pt, e, ib, 5)
                if vp0:
                    for _, rc in vp0[3:]:
                        vp_block(vcb, 0, rc)
                if t < last:
                    vp_block(vcb_cur, t + 1, 4 * idx + 2)
                    vp_block(vcb_cur, t + 1, 4 * idx + 3)
                score_block(ktp, qtp, pt, e, ib, 6)
                score_block(ktp, qtp, pt, e, ib, 7)
                if t < last and idx in (0, 2):
                    qp_tile(t + 1, idx // 2, qtp_cur)
                if t == last - 1 and idx == 3:
                    # wot reuses wq's slot; wq's last read was qp_tile above
                    wot_sb = load_w_ot()
                # older blocks' PV / normalize / transpose.  Queue depth 2
                # (pt pool holds 3) absorbs pair 0's double projection load;
                # pair 7 drains to depth 1 so the tail stays short.
                while len(pendings) >= (1 if t == last else 2):
                    emit_pv()
                pendings.append((t, e, ib, pt, vcb))
                # pair 7 (ib-major): overlap the ib=0 half of the output
                # projection with the remaining attention blocks
                if t == last and (e, ib) == (0, 1):
                    for rc8 in (0, 1):
                        op_slice(rc8)
                if t == last and (e, ib) == (1, 1):
                    for rc8 in (2, 3):
                        op_slice(rc8)
        # tail: last block's PV interleaved with the ib=1 out projection
        t, e, ib, pt, vcb = pendings.pop(0)
        for qc in range(NB // P):
            pv_chain(t, e, ib, pt, vcb, qc)
            op_slice(QR // P // 2 + qc)

    import bass_rust as _bass_rust
    from concourse.library_config import all_libraries, standard

    inst_type_to_lib_mask = {}
    for lib in all_libraries:
        for inst_type in lib.instructions:
            inst_type_to_lib_mask[inst_type] = inst_type_to_lib_mask.get(
                inst_type, 0) | (1 << lib.index)
    _bass_rust.insert_library_loads(
        nc, inst_type_to_lib_mask, len(all_libraries), standard.index)

    _split_multi_waits(nc, mybir)

    return nc


def _split_multi_waits(nc, mybir):
    """This walrus build accepts at most ONE sync-wait per instruction; Tile
    emits several.  Hoist all but the last wait onto single-wait NoOps placed
    immediately before the instruction on the same engine."""
    nop_id = [0]
    for fn in nc.m.functions:
        for bb in fn.blocks:
            out = []
            for inst in bb.instructions:
                si = inst.sync_info
                if si is not None and si.on_wait is not None \
                        and len(si.on_wait) > 1:
                    waits = list(si.on_wait)
                    for w in waits[:-1]:
                        nop = mybir.InstNoOp(
                            name=f"I-waitsplit-{nop_id[0]}", ins=[], outs=[])
                        nop_id[0] += 1
                        nop.engine = inst.engine
                        nop.sync_info = mybir.SyncInfo(
                            on_wait=[w], on_update=[])
                        out.append(nop)
                    inst.sync_info = mybir.SyncInfo(
                        on_wait=[waits[-1]],
                        on_update=list(si.on_update or []))
                out.append(inst)
            bb.instructions = out


def _get_compiled():
    global _COMPILED
    if _COMPILED is None:
        _COMPILED = _build()
    return _COMPILED


def kernel(x, wq, bq, wk, bk, wv, bv, wo, bo, _want_results_obj=False,
           **run_kwargs):
    import ml_dtypes
    from concourse.bass_utils import run_bass_kernel_spmd

    x = np.asarray(x, dtype=np.float32)
    wq = np.asarray(wq, dtype=np.float32)
    bq = np.asarray(bq, dtype=np.float32)
    wk = np.asarray(wk, dtype=np.float32)
    bk = np.asarray(bk, dtype=np.float32)
    wv = np.asarray(wv, dtype=np.float32)
    bv = np.asarray(bv, dtype=np.float32)
    wo = np.asarray(wo, dtype=np.float32)
    bo = np.asarray(bo, dtype=np.float32)

    bs, seq, emb = x.shape
    assert (bs, seq, emb) == (4, SEQ, EMB)

    nc = _get_compiled()

    bf16 = ml_dtypes.bfloat16
    shared = {
        "wqt": np.ascontiguousarray(wq.T).astype(bf16),
        "wkt": np.ascontiguousarray(wk.T).astype(bf16),
        "wvt": np.ascontiguousarray(wv.T).astype(bf16),
        "wot": np.ascontiguousarray(wo.T).astype(bf16),
        "bqp": np.ascontiguousarray(bq.reshape(EC, P).T),
        "bkp": np.ascontiguousarray(bk.reshape(EC, P).T),
        "bob": np.ascontiguousarray(
            np.broadcast_to(bo + wo @ bv, (P, EMB))),
        "idn": np.eye(P, dtype=bf16),
    }
    in_maps = []
    for c in range(8):
        b, hf = c // 2, c % 2
        xb = x[b]
        # this core's query rows first; row order of keys/values is irrelevant
        xb_perm = np.concatenate(
            [xb[hf * QR:(hf + 1) * QR], xb[(1 - hf) * QR:(2 - hf) * QR]], axis=0)
        in_maps.append({
            "xt": np.ascontiguousarray(xb_perm.T).astype(bf16),
            **shared,
        })

    res = run_bass_kernel_spmd(nc, in_maps, core_ids=list(range(8)),
                               **run_kwargs)

    outp = np.empty((bs, seq, emb), dtype=np.float32)
    for c in range(8):
        b, hf = c // 2, c % 2
        outp[b, hf * QR:(hf + 1) * QR, :] = res.results[c]["out"]
    if _want_results_obj:
        return outp, res
    return outp


# revision 21
# speedup vs baseline: 1.0137x; 1.0137x over previous
"""Multi-head attention kernel for 8 TRN2 NeuronCores.

Problem: x(4,2048,1024) -> MHA(16 heads, d=64) -> out(4,2048,1024), f32.

Sharding: core c handles (batch b = c//2, seq half = c%2): it computes
attention outputs (incl. all projections) for its 1024 query rows over all 16
heads.  K/V projections for the full batch are computed locally per core (2x
redundant) which keeps cores fully independent - zero collectives.

v5 highlights:

* All projections run in bf16 (x and weights cast on host).
* Scores run in fp8e4 with the DoubleRow perf mode (0.5 cycles/moving-row):
  the pair dim carries a DUAL-fp8 decomposition of Q: pair0 = (q_hi, k8),
  pair1 = (q_lo, k8/4) where q_lo = fp8(4*(q - q_hi)), so the PE computes
  k8^T (q_hi + q_lo/4) - near-bf16 Q precision at fp8 speed.  K stays
  single-fp8 (~7e-3 rel err end to end).
* Softmax exp runs on ScalarE out of PSUM, writing bf16.  ScalarE is the
  bottleneck engine (~266us busy), so the schedule keeps its queue non-empty
  from the first scores (~16us) to the end.
* PV runs in bf16 in the SWAPPED orientation: ov[q,65] = E^T @ [V | 1];
  column 64 (ones) gives the softmax denominator per q ROW -> normalization
  is a partition-local reciprocal + tensor_scalar_mul.  The normalized
  [128q, 64d] tile is PE-transposed (bf16) back into att_sb[emb, q] for the
  bf16 output projection.
* Q and V never touch DRAM: Q-projection evacs write dual-fp8 directly into
  the next pair's qtp SBUF tile; the V projection is computed
  head-pair-column-major into SBUF-resident vcb tiles laid out as
  [keys, rc x (64 V cols even | 1.0 | 64 V cols odd | 1.0)] which the PV
  matmuls read directly.
* Software pipelining: per head-pair iteration the emission interleaves this
  block's scores (early jc2 first, matching ScalarE's drain rate) with the
  next pair's K/V/Q projection blocks, then emits the PREVIOUS block's
  PV/normalize/transpose, so the in-order PE always has work that does not
  wait on ScalarE.  The output projection is split into slices overlapped
  with the last pair's attention and the final PV chunks.

This walrus build accepts only ONE sync-wait per instruction, so a post-pass
splits multi-wait instructions into single-wait NoOps (_split_multi_waits).
"""

import numpy as np
from contextlib import ExitStack

P = 128
EMB = 1024
SEQ = 2048
QR = 1024          # query rows per core
NH = 16
HD = 64
EC = EMB // P      # 8 contraction chunks
RC = SEQ // P      # 16 seq row chunks
NB = 512           # free-dim block
SCALE = 0.125      # 1/sqrt(64)

_COMPILED = None


def _patch_tile_drain():
    """This walrus build only accepts ONE sync-wait per Drain instruction; the
    stock TileContext tail drain carries one wait per pending proc.  Split it
    into a chain of single-wait drains."""
    import concourse.tile as tile
    from concourse.vector_clock import ScopedClock, VectorClock

    if getattr(tile.TileContext, "_drain_patched", False):
        return

    def _drain_and_barrier(self, tick_clock, wait_clock):
        nc = self.nc
        gc = tick_clock.global_clock
        vals = eval(repr(gc).replace("VectorClock", ""))
        n = len(vals)
        for i, v in enumerate(vals):
            if v > 0:
                sub = VectorClock([vals[j] if j == i else 0 for j in range(n)])
                d = nc.sync.drain()
                wait_clock.add_sem_waits(d.ins, ScopedClock({None: sub}))
        nc.all_engine_barrier()
        popped = nc._tile_sem_poison_stack.pop()
        assert popped is self._sem_poison
        nc.clear_and_free_semaphores(list(self.sems.allocated().values()))
        nc.all_engine_barrier()

    tile.TileContext._drain_and_barrier = _drain_and_barrier
    tile.TileContext._drain_patched = True


def _build():
    import concourse.bass as bass
    import concourse.mybir as mybir
    import concourse.tile as tile

    _patch_tile_drain()

    f32 = mybir.dt.float32
    f8 = mybir.dt.float8e4
    bf16 = mybir.dt.bfloat16
    Exp = mybir.ActivationFunctionType.Exp
    DR = mybir.MatmulPerfMode.DoubleRow
    Sub = mybir.AluOpType.subtract

    nc = bass.Bass()

    # xt holds this core's batch transposed, with the core's 1024 query rows
    # FIRST (host pre-permutes; key/value row order is irrelevant to MHA).
    xt = nc.dram_tensor("xt", [EMB, SEQ], bf16, kind="ExternalInput")
    wqt = nc.dram_tensor("wqt", [EMB, EMB], bf16, kind="ExternalInput")
    wkt = nc.dram_tensor("wkt", [EMB, EMB], bf16, kind="ExternalInput")
    wvt = nc.dram_tensor("wvt", [EMB, EMB], bf16, kind="ExternalInput")
    wot = nc.dram_tensor("wot", [EMB, EMB], bf16, kind="ExternalInput")
    bqp = nc.dram_tensor("bqp", [P, EC], f32, kind="ExternalInput")
    bkp = nc.dram_tensor("bkp", [P, EC], f32, kind="ExternalInput")
    bob = nc.dram_tensor("bob", [P, EMB], f32, kind="ExternalInput")
    idn = nc.dram_tensor("idn", [P, P], bf16, kind="ExternalInput")
    out = nc.dram_tensor("out", [QR, EMB], f32, kind="ExternalOutput")

    with tile.TileContext(nc) as tc, ExitStack() as ctx:
        big = ctx.enter_context(tc.tile_pool(name="big", bufs=1))
        wpool = ctx.enter_context(tc.tile_pool(name="w", bufs=1))
        pspool = ctx.enter_context(tc.tile_pool(name="ps", bufs=2, space="PSUM"))
        stpool = ctx.enter_context(tc.tile_pool(name="st", bufs=2, space="PSUM"))
        ovpool = ctx.enter_context(tc.tile_pool(name="ov", bufs=2, space="PSUM"))
        evac = ctx.enter_context(tc.tile_pool(name="evac", bufs=3))
        ptpool = ctx.enter_context(tc.tile_pool(name="pt", bufs=3))
        kpool = ctx.enter_context(tc.tile_pool(name="kp", bufs=2))
        wkpool = ctx.enter_context(tc.tile_pool(name="wk", bufs=2))
        qpool = ctx.enter_context(tc.tile_pool(name="qp", bufs=2))
        vpool = ctx.enter_context(tc.tile_pool(name="vp", bufs=3))
        nrm = ctx.enter_context(tc.tile_pool(name="nrm", bufs=2))
        misc = ctx.enter_context(tc.tile_pool(name="misc", bufs=1))

        # ---- persistent loads (critical-path order) ---------------------
        bq_sb = misc.tile([P, EC], f32, tag="bq")
        nc.sync.dma_start(bq_sb[:], bqp[:])
        bk_sb = misc.tile([P, EC], f32, tag="bk")
        nc.sync.dma_start(bk_sb[:], bkp[:])

        def load_wk(t):
            wk_t = wkpool.tile([P, EC * P], bf16, tag="wk", name="wk_t")
            for ec in range(EC):
                nc.sync.dma_start(
                    wk_t[:, ec * P:(ec + 1) * P],
                    wkt[ec * P:(ec + 1) * P, t * P:(t + 1) * P])
            return wk_t

        wk_tiles = {0: load_wk(0)}
        # wq and x query columns interleaved per ec chunk so the first
        # Q-projection chain pipelines behind the DMA
        wq_sb = wpool.tile([P, EC * EMB], bf16, tag="w", name="wq_sb")
        xt_sb = big.tile([P, EC * SEQ], bf16, tag="xt")
        for ec in range(EC):
            nc.sync.dma_start(wq_sb[:, ec * EMB:(ec + 1) * EMB],
                              wqt[ec * P:(ec + 1) * P, :])
            nc.sync.dma_start(xt_sb[:, ec * SEQ: ec * SEQ + QR],
                              xt[ec * P:(ec + 1) * P, 0:QR])
        for ec in range(EC):
            nc.sync.dma_start(xt_sb[:, ec * SEQ + QR: (ec + 1) * SEQ],
                              xt[ec * P:(ec + 1) * P, QR:SEQ])

        # ---- building blocks --------------------------------------------
        def qp_tile(oc, ib, qtp_dst):
            """Q projection for output chunk oc (= head pair), query block ib,
            written as dual-fp8 (q_hi at [ib*NB], q_lo at [QR+ib*NB])
            directly into the pair's qtp SBUF tile."""
            ps = pspool.tile([P, NB], f32, tag="ps", name="ps")
            for ec in range(EC):
                nc.tensor.matmul(
                    ps[:],
                    wq_sb[:, ec * EMB + oc * P: ec * EMB + (oc + 1) * P],
                    xt_sb[:, ec * SEQ + ib * NB: ec * SEQ + (ib + 1) * NB],
                    start=(ec == 0), stop=(ec == EC - 1))
            qf = evac.tile([P, NB], f32, tag="qf", name="qf")
            nc.vector.tensor_scalar_add(qf[:], ps[:], bq_sb[:, oc:oc + 1])
            nc.vector.tensor_copy(qtp_dst[:, ib * NB:(ib + 1) * NB], qf[:])
            qd = evac.tile([P, NB], f32, tag="qd", name="qd")
            nc.vector.tensor_tensor(
                qd[:], qf[:], qtp_dst[:, ib * NB:(ib + 1) * NB], Sub)
            nc.vector.tensor_scalar_mul(
                qtp_dst[:, QR + ib * NB: QR + (ib + 1) * NB], qd[:], 4.0)

        def kp_block(ktp, t, jb):
            """K projection for head pair t, key block jb -> fp8 K in
            ktp[:, 0:SEQ] and fp8 K/4 in ktp[:, SEQ:2SEQ] (DoubleRow pair
            partner for the dual-fp8 Q correction term)."""
            ps = pspool.tile([P, NB], f32, tag="ps", name="ps")
            for ec in range(EC):
                nc.tensor.matmul(
                    ps[:],
                    wk_tiles[t][:, ec * P:(ec + 1) * P],
                    xt_sb[:, ec * SEQ + jb * NB: ec * SEQ + (jb + 1) * NB],
                    start=(ec == 0), stop=(ec == EC - 1))
            nc.vector.tensor_scalar_add(
                ktp[:, jb * NB:(jb + 1) * NB], ps[:], bk_sb[:, t:t + 1])
            nc.vector.tensor_scalar(
                ktp[:, SEQ + jb * NB: SEQ + (jb + 1) * NB], ps[:],
                bk_sb[:, t:t + 1], 0.25,
                mybir.AluOpType.add, mybir.AluOpType.mult)

        def vp_block(vcb, t, rc):
            """V projection for head pair t, key row chunk rc, written into
            the SBUF-resident [V_even | 1 | V_odd | 1] layout."""
            ps = pspool.tile([P, P], f32, tag="ps", name="ps")
            for ec in range(EC):
                nc.tensor.matmul(
                    ps[:],
                    xt_sb[:, ec * SEQ + rc * P: ec * SEQ + (rc + 1) * P],
                    wv_sb[:, ec * EMB + t * P: ec * EMB + (t + 1) * P],
                    start=(ec == 0), stop=(ec == EC - 1))
            nc.vector.tensor_copy(vcb[:, rc * 130: rc * 130 + HD], ps[:, 0:HD])
            nc.vector.tensor_copy(vcb[:, rc * 130 + 65: rc * 130 + 65 + HD],
                                  ps[:, HD:P])

        def new_vcb():
            vcb = vpool.tile([P, RC * 130], bf16, tag="vcb", name="vcb")
            for e in range(2):
                ones_ap = bass.AP(vcb.tensor, vcb.offset + e * 65 + HD,
                                  [list(vcb.ap[0]), [130, RC]])
                nc.vector.memset(ones_ap, 1.0)
            return vcb

        def op_slice(rc8):
            for ob in range(2):
                ps = pspool.tile([P, NB], f32, tag="ps", name="ps")
                for cc in range(EC):
                    nc.tensor.matmul(
                        ps[:],
                        att_sb[:, cc * QR + rc8 * P: cc * QR + (rc8 + 1) * P],
                        wot_sb[:, cc * EMB + ob * NB: cc * EMB + (ob + 1) * NB],
                        start=(cc == 0), stop=(cc == EC - 1))
                ev = evac.tile([P, NB], f32, tag="evo", name="ev")
                nc.vector.tensor_add(
                    ev[:], ps[:], bob_sb[:, ob * NB:(ob + 1) * NB])
                nc.gpsimd.dma_start(
                    out[rc8 * P:(rc8 + 1) * P, ob * NB:(ob + 1) * NB], ev[:])

        def pair_ap(t2, pstart, pcount, off, pair_stride, n):
            """[pcount partitions from pstart] x (2 pairs) x (n contiguous)."""
            sl = t2[pstart:pstart + pcount]
            return bass.AP(sl.tensor, sl.offset + off,
                           [list(sl.ap[0]), [pair_stride, 2], [1, n]])

        def score_block(ktp, qtp, pt, e, ib, jc2):
            st_ps = stpool.tile([P, 2 * NB], f32, tag="st")
            for u in range(2):
                jc = jc2 * 2 + u
                nc.tensor.matmul(
                    st_ps[:, u * NB:(u + 1) * NB],
                    pair_ap(ktp, e * HD, HD, jc * P, SEQ, P),
                    pair_ap(qtp, e * HD, HD, ib * NB, QR, NB),
                    start=True, stop=True, perf_mode=DR)
            nc.scalar.activation(pt[:, jc2 * 2 * NB:(jc2 + 1) * 2 * NB],
                                 st_ps[:], Exp, scale=SCALE)

        pendings = []

        def pv_chain(t, e, ib, pt, vcb, qc):
            ov = ovpool.tile([P, 65], f32, tag="ov", name="ov")
            for rc in range(RC):
                jc2, u = rc // 2, rc % 2
                nc.tensor.matmul(
                    ov[:],
                    pt[:, jc2 * 2 * NB + u * NB + qc * P:
                       jc2 * 2 * NB + u * NB + (qc + 1) * P],
                    vcb[:, rc * 130 + e * 65: rc * 130 + (e + 1) * 65],
                    start=(rc == 0), stop=(rc == RC - 1))
            rs = nrm.tile([P, 1], f32, tag="rs")
            nc.vector.reciprocal(rs[:], ov[:, HD:65])
            atmp = nrm.tile([P, HD], bf16, tag="atmp")
            nc.vector.tensor_scalar_mul(atmp[:], ov[:, 0:HD], rs[:, 0:1])
            trp = ovpool.tile([HD, P], bf16, tag="ov", name="trp")
            nc.tensor.transpose(trp[:], atmp[:], idn_sb[:])
            nc.vector.tensor_copy(
                att_sb[e * HD:(e + 1) * HD,
                       t * QR + ib * NB + qc * P:
                       t * QR + ib * NB + (qc + 1) * P],
                trp[:])

        def emit_pv():
            t, e, ib, pt, vcb = pendings.pop(0)
            for qc in range(NB // P):
                pv_chain(t, e, ib, pt, vcb, qc)

        # keep-warm tile for pair 7's light blocks (dummy matmuls keep the PE
        # p-state at full clock ahead of the tail output projection)
        wtile = misc.tile([P, NB], bf16, tag="warm")
        nc.vector.memset(wtile[:], 1.0)

        # ---- preamble: pair 0's inputs ----------------------------------
        qtp_cur = qpool.tile([P, 2 * QR], f8, tag="qt", name="qtp")
        qp_tile(0, 0, qtp_cur)
        ktp_cur = kpool.tile([P, 2 * SEQ], f8, tag="kt", name="ktp")
        kp_block(ktp_cur, 0, 0)
        kp_block(ktp_cur, 0, 1)
        qp_tile(0, 1, qtp_cur)
        kp_block(ktp_cur, 0, 2)
        kp_block(ktp_cur, 0, 3)

        # non-critical loads: wv, out-proj bias, identity
        wv_sb = big.tile([P, EC * EMB], bf16, tag="wv", name="wv_sb")
        for ec in range(EC):
            nc.sync.dma_start(wv_sb[:, ec * EMB:(ec + 1) * EMB],
                              wvt[ec * P:(ec + 1) * P, :])
        bob_sb = misc.tile([P, EMB], f32, tag="bob")
        nc.sync.dma_start(bob_sb[:], bob[:])
        idn_sb = misc.tile([P, P], bf16, tag="idn")
        nc.sync.dma_start(idn_sb[:], idn[:])

        # pair 0's V projection is NOT done here: its 16 chains would sit on
        # the PE critical path before the first scores and delay ScalarE's
        # start by ~7us.  It is interleaved into pair 0's first two blocks
        # instead (PV needs it only from the second block's end).
        vcb_cur = new_vcb()

        att_sb = big.tile([P, EC * QR], bf16, tag="att")
        wot_sb = None

        def load_w_ot():
            w_sb = wpool.tile([P, EC * EMB], bf16, tag="w", name="w_sb")
            for ec in range(EC):
                nc.sync.dma_start(w_sb[:, ec * EMB:(ec + 1) * EMB],
                                  wot[ec * P:(ec + 1) * P, :])
            return w_sb

        # ---- main loop: 8 head pairs, fully pipelined -------------------
        last = NH // 2 - 1
        for t in range(NH // 2):
            ktp, qtp, vcb = ktp_cur, qtp_cur, vcb_cur
            if t < last:
                wk_tiles[t + 1] = load_wk(t + 1)
                ktp_cur = kpool.tile([P, 2 * SEQ], f8, tag="kt", name="ktp")
                vcb_cur = new_vcb()
                qtp_cur = qpool.tile([P, 2 * QR], f8, tag="qt", name="qtp")
            ebs = ([(e, ib) for e in range(2) for ib in range(QR // NB)]
                   if t < last else
                   [(e, ib) for ib in range(QR // NB) for e in range(2)])
            for idx, (e, ib) in enumerate(ebs):
                h = 2 * t + e
                pt = ptpool.tile([P, RC * NB], bf16, tag="pt")
                # scores early (refill ScalarE), next-pair projections
                # interleaved at ScalarE's drain rate
                score_block(ktp, qtp, pt, e, ib, 0)
                score_block(ktp, qtp, pt, e, ib, 1)
                if t < last:
                    kp_block(ktp_cur, t + 1, idx)
                score_block(ktp, qtp, pt, e, ib, 2)
                score_block(ktp, qtp, pt, e, ib, 3)
                vp0 = ([(0, rc) for rc in range(8 * idx, 8 * idx + 8)]
                       if t == 0 and idx < 2 else [])
                if vp0:
                    # pair 0's own V projection (deferred from the preamble)
                    for _, rc in vp0[:4]:
                        vp_block(vcb, 0, rc)
                if t < last:
                    vp_block(vcb_cur, t + 1, 4 * idx)
                    vp_block(vcb_cur, t + 1, 4 * idx + 1)
                score_block(ktp, qtp, pt, e, ib, 4)
                score_block(ktp, qtp, pt, e, ib, 5)
                if vp0:
                    for _, rc in vp0[4:]:
                        vp_block(vcb, 0, rc)
                if t < last:
                    vp_block(vcb_cur, t + 1, 4 * idx + 2)
                    vp_block(vcb_cur, t + 1, 4 * idx + 3)
                score_block(ktp, qtp, pt, e, ib, 6)
                score_block(ktp, qtp, pt, e, ib, 7)
                if t < last and idx in (0, 2):
                    qp_tile(t + 1, idx // 2, qtp_cur)
                if t == last - 1 and idx == 3:
                    # wot reuses wq's slot; wq's last read was qp_tile above
                    wot_sb = load_w_ot()
                # previous block's PV / normalize / transpose
                while len(pendings) >= 1:
                    emit_pv()
                pendings.append((t, e, ib, pt, vcb))
                if t == last and idx < 2:
                    # keep the PE p-state warm through pair 7's light blocks
                    # so the tail output projection runs at full clock
                    for _ in range(8):
                        ps = pspool.tile([P, NB], f32, tag="ps", name="ps")
                        nc.tensor.matmul(ps[:], wtile[:, 0:P], wtile[:],
                                         start=True, stop=True)
                # pair 7 (ib-major): overlap the ib=0 half of the output
                # projection with the remaining attention blocks
                if t == last and (e, ib) == (0, 1):
                    for rc8 in (0, 1):
                        op_slice(rc8)
                if t == last and (e, ib) == (1, 1):
                    for rc8 in (2, 3):
                        op_slice(rc8)
        # tail: last block's PV interleaved with the ib=1 out projection
        t, e, ib, pt, vcb = pendings.pop(0)
        for qc in range(NB // P):
            pv_chain(t, e, ib, pt, vcb, qc)
            op_slice(QR // P // 2 + qc)

    import bass_rust as _bass_rust
    from concourse.library_config import all_libraries, standard

    inst_type_to_lib_mask = {}
    for lib in all_libraries:
        for inst_type in lib.instructions:
            inst_type_to_lib_mask[inst_type] = inst_type_to_lib_mask.get(
                inst_type, 0) | (1 << lib.index)
    _bass_rust.insert_library_loads(
        nc, inst_type_to_lib_mask, len(all_libraries), standard.index)

    _split_multi_waits(nc, mybir)

    return nc


def _split_multi_waits(nc, mybir):
    """This walrus build accepts at most ONE sync-wait per instruction; Tile
    emits several.  Hoist all but the last wait onto single-wait NoOps placed
    immediately before the instruction on the same engine."""
    nop_id = [0]
    for fn in nc.m.functions:
        for bb in fn.blocks:
            out = []
            for inst in bb.instructions:
                si = inst.sync_info
                if si is not None and si.on_wait is not None \
                        and len(si.on_wait) > 1:
                    waits = list(si.on_wait)
                    for w in waits[:-1]:
                        nop = mybir.InstNoOp(
                            name=f"I-waitsplit-{nop_id[0]}", ins=[], outs=[])
                        nop_id[0] += 1
                        nop.engine = inst.engine
                        nop.sync_info = mybir.SyncInfo(
                            on_wait=[w], on_update=[])
                        out.append(nop)
                    inst.sync_info = mybir.SyncInfo(
                        on_wait=[waits[-1]],
                        on_update=list(si.on_update or []))
                out.append(inst)
            bb.instructions = out


def _get_compiled():
    global _COMPILED
    if _COMPILED is None:
        _COMPILED = _build()
    return _COMPILED


def kernel(x, wq, bq, wk, bk, wv, bv, wo, bo, _want_results_obj=False,
           **run_kwargs):
    import ml_dtypes
    from concourse.bass_utils import run_bass_kernel_spmd

    x = np.asarray(x, dtype=np.float32)
    wq = np.asarray(wq, dtype=np.float32)
    bq = np.asarray(bq, dtype=np.float32)
    wk = np.asarray(wk, dtype=np.float32)
    bk = np.asarray(bk, dtype=np.float32)
    wv = np.asarray(wv, dtype=np.float32)
    bv = np.asarray(bv, dtype=np.float32)
    wo = np.asarray(wo, dtype=np.float32)
    bo = np.asarray(bo, dtype=np.float32)

    bs, seq, emb = x.shape
    assert (bs, seq, emb) == (4, SEQ, EMB)

    nc = _get_compiled()

    bf16 = ml_dtypes.bfloat16
    shared = {
        "wqt": np.ascontiguousarray(wq.T).astype(bf16),
        "wkt": np.ascontiguousarray(wk.T).astype(bf16),
        "wvt": np.ascontiguousarray(wv.T).astype(bf16),
        "wot": np.ascontiguousarray(wo.T).astype(bf16),
        "bqp": np.ascontiguousarray(bq.reshape(EC, P).T),
        "bkp": np.ascontiguousarray(bk.reshape(EC, P).T),
        "bob": np.ascontiguousarray(
            np.broadcast_to(bo + wo @ bv, (P, EMB))),
        "idn": np.eye(P, dtype=bf16),
    }
    in_maps = []
    for c in range(8):
        b, hf = c // 2, c % 2
        xb = x[b]
        # this core's query rows first; row order of keys/values is irrelevant
        xb_perm = np.concatenate(
            [xb[hf * QR:(hf + 1) * QR], xb[(1 - hf) * QR:(2 - hf) * QR]], axis=0)
        in_maps.append({
            "xt": np.ascontiguousarray(xb_perm.T).astype(bf16),
            **shared,
        })

    res = run_bass_kernel_spmd(nc, in_maps, core_ids=list(range(8)),
                               **run_kwargs)

    outp = np.empty((bs, seq, emb), dtype=np.float32)
    for c in range(8):
        b, hf = c // 2, c % 2
        outp[b, hf * QR:(hf + 1) * QR, :] = res.results[c]["out"]
    if _want_results_obj:
        return outp, res
    return outp
